# revision 1
# baseline (speedup 1.0000x reference)
"""GQA attention kernel for Trainium2, sharded over 8 NeuronCores.

Problem: X (1, 4096, 1024), H=16 q-heads, KVH=4 kv-heads, head_dim=64.
Sharding: 2 q-heads + their shared kv-head per core (tensor parallel over H).
Each core computes q/k/v projections for its heads, fused flash-style
attention, and its 128-row slice of the output projection -> partial
(4096, 1024) f32, summed on host.

v2: software-pipelined single instruction stream tuned to keep the PE
array continuously busy (it only reaches the 2.4 GHz p-state after ~3us
without stalls; the v1 kernel sat at 1.2 GHz through all of attention):
  - scores(t) are emitted before PV(t-1), so exp(t-1) on the Scalar
    engine overlaps score matmuls and PV never waits on it.
  - KV projection (chunk 0), Q projection (chunk c+1) and the output
    projection (chunk c-1) are spread through the attention loop as PE
    filler work instead of serial prologue/epilogue phases.
  - softmax normalization broadcasts the reciprocal denominators with a
    GpSimd partition_broadcast (v1 round-tripped through DRAM, stalling
    the PE ~20us per chunk).

Layouts on device (per core):
  xt   : X^T            (1024 D, 4096 S)  bf16   (host pre-transposed)
  qt   : Q^T            (128 = 2 heads x 64 d, 4096 q) bf16
  kvt  : [K^T; V^T]     (128 = 64 k-d + 64 v-d, 4096 s) bf16
  kt2  : K^T duplicated into both partition halves
  v    : V natural+ones (128 s-tile, 65) x 32 tiles bf16 (col 64 == 1.0)
  St   : scores^T       (128 k, 1024 q) f32 PSUM  = Kt_tile.T @ Qt
  Pt   : exp(St/8)      (128 k, 1024 q) bf16 SBUF (ScalarE, scale folded)
  Ot   : V_aug.T @ Pt   (65, 1024) f32 PSUM; row 64 = softmax denominators
  y    : partial output (4096, 1024) f32 = (Ot/denoms).T @ o_w[rows]
"""

import sys

import numpy as np

try:
    import concourse.bass as bass
except ImportError:  # grading env may not have concourse on sys.path
    for p in ("/opt/trn_rl_repo", "/root/.axon_site/_ro/trn_rl_repo"):
        if p not in sys.path:
            sys.path.append(p)
    import concourse.bass as bass

import bass_rust
import ml_dtypes
from concourse import mybir
from concourse.bass_utils import run_bass_kernel_spmd
from concourse.masks import make_identity
from concourse.tile import TileContext

BF16 = ml_dtypes.bfloat16

B, S, D = 1, 4096, 1024
H, KVH, HD = 16, 4, 64
NCORES = 8
HPC = H // NCORES          # 2 q heads per core
DQ = HPC * HD              # 128 projected q dims per core
DKV = 2 * HD               # 128 = k head + v head dims
QC = 1024                  # attention q-chunk (2 PSUM banks per score tile)
KT = 128                   # k tile (seq positions per score tile)
NKT = S // KT              # 32
NQC = S // QC              # 4
NDC = D // 128             # 8 contraction chunks for projections
MM_N = 512                 # max matmul free dim (one PSUM bank, f32)

_COMPILED = None


def build_bass():
    nc = bass.Bass()
    fp32 = mybir.dt.float32
    bf16 = mybir.dt.bfloat16

    xt = nc.declare_dram_parameter("xt", [D, S], bf16, isOutput=False)
    qw = nc.declare_dram_parameter("qw", [D, DQ], bf16, isOutput=False)
    kvw = nc.declare_dram_parameter("kvw", [D, DKV], bf16, isOutput=False)
    ow = nc.declare_dram_parameter("ow", [DQ, D], bf16, isOutput=False)
    qb = nc.declare_dram_parameter("qb", [DQ, 1], fp32, isOutput=False)
    kvb = nc.declare_dram_parameter("kvb", [DKV, 1], fp32, isOutput=False)
    y = nc.declare_dram_parameter("y", [S, D], bf16, isOutput=True)

    with TileContext(nc) as tc:
        with (
            tc.tile_pool(name="singles", bufs=1) as singles,
            tc.tile_pool(name="pt_pool", bufs=8) as pt_pool,
            tc.tile_pool(name="nrm", bufs=2) as nrm_pool,
            tc.tile_pool(name="otcp", bufs=2) as otcp_pool,
            tc.tile_pool(name="ysb", bufs=8) as ysb_pool,
            tc.tile_pool(name="ps_st", bufs=2, space="PSUM") as ps_st,
            tc.tile_pool(name="ps_ot", bufs=2, space="PSUM") as ps_ot,
        ):
            # ---- constants / weights ----
            ident = singles.tile([128, 128], bf16)
            make_identity(nc, ident)

            # DMA order matters: the prologue needs xt slices 0-1 and the
            # q/kv weights first; the remaining xt slices pace the chunk-0
            # kv fillers; ow is only needed ~100us in, so it goes last.
            # xt moves as 8 DMAs of [128 part, 4 c-chunks, 1024 cols]: 2 KB
            # contiguous DRAM lines (vs 1 KB at 512 cols) for better
            # per-queue throughput; weights split 2 ways to land early.
            xt_sb = singles.tile([128, NDC, S], bf16)
            xt_re = xt[:, :].rearrange("(c p) s -> p c s", p=128)
            kvw_sb = singles.tile([128, NDC, DKV], bf16)
            kvw_re = kvw[:, :].rearrange("(c p) m -> p c m", p=128)
            qw_sb = singles.tile([128, NDC, DQ], bf16)
            qw_re = qw[:, :].rearrange("(c p) m -> p c m", p=128)
            for half in range(2):
                nc.sync.dma_start(
                    out=kvw_sb[:, bass.ts(half, 4), :],
                    in_=kvw_re[:, bass.ts(half, 4), :],
                )
                nc.sync.dma_start(
                    out=qw_sb[:, bass.ts(half, 4), :],
                    in_=qw_re[:, bass.ts(half, 4), :],
                )
            qb_sb = singles.tile([DQ, 1], fp32)
            nc.sync.dma_start(out=qb_sb, in_=qb[:, :])
            kvb_sb = singles.tile([DKV, 1], fp32)
            nc.sync.dma_start(out=kvb_sb, in_=kvb[:, :])
            for j in range(S // QC):
                for ch in range(2):
                    nc.sync.dma_start(
                        out=xt_sb[:, bass.ts(ch, 4), bass.ts(j, QC)],
                        in_=xt_re[:, bass.ts(ch, 4), bass.ts(j, QC)],
                    )
            ow_sb = singles.tile([DQ, D], bf16)
            nc.sync.dma_start(out=ow_sb, in_=ow[:, :])

            qt_sb = singles.tile([DQ, S], bf16)
            kvt_sb = singles.tile([DKV, S], bf16)
            kt2_sb = singles.tile([DKV, S], bf16)
            v_sb = singles.tile([128, NKT, HD + 1], bf16)
            nc.vector.memset(v_sb, 1.0)
            ot_full = singles.tile([DQ, S], bf16)
            ones_col = singles.tile([1, HD], bf16)
            nc.vector.memset(ones_col, 1.0)

            exp = mybir.ActivationFunctionType.Exp

            def proj_slice(dst, w_sb, b_sb, j):
                ps = ps_st.tile([128, MM_N], fp32, tag="st")
                for c2 in range(NDC):
                    nc.tensor.matmul(
                        ps, w_sb[:, c2, :], xt_sb[:, c2, bass.ts(j, MM_N)],
                        start=(c2 == 0), stop=(c2 == NDC - 1),
                    )
                nc.vector.tensor_scalar_add(
                    dst[:, bass.ts(j, MM_N)], ps, b_sb[:, 0:1]
                )

            def kv_slice(j):
                proj_slice(kvt_sb, kvw_sb, kvb_sb, j)
                nc.sync.dma_start(
                    out=kt2_sb[0:HD, bass.ts(j, MM_N)],
                    in_=kvt_sb[0:HD, bass.ts(j, MM_N)],
                )
                nc.sync.dma_start(
                    out=kt2_sb[HD:DKV, bass.ts(j, MM_N)],
                    in_=kvt_sb[0:HD, bass.ts(j, MM_N)],
                )
                for tt in range(4 * j, 4 * j + 4):
                    pvt = ps_st.tile([128, HD], bf16, tag="st")
                    nc.tensor.transpose(
                        pvt, kvt_sb[HD:DKV, bass.ts(tt, KT)],
                        ident[HD:DKV, HD:DKV],
                    )
                    nc.vector.tensor_copy(v_sb[:, tt, 0:HD], pvt)

            def emit_scores(c, t):
                res = []
                for h in range(HPC):
                    st = ps_st.tile([128, QC], fp32, tag="st")
                    for u in range(QC // MM_N):
                        nc.tensor.matmul(
                            st[:, bass.ts(u, MM_N)],
                            kt2_sb[h * HD:(h + 1) * HD, bass.ts(t, KT)],
                            qt_sb[h * HD:(h + 1) * HD,
                                  c * QC + u * MM_N:c * QC + (u + 1) * MM_N],
                            start=True, stop=True,
                        )
                    pt = pt_pool.tile([128, QC], bf16, tag="pt")
                    nc.scalar.activation(pt, st, exp, scale=1.0 / 8.0)
                    res.append(pt)
                return res

            ots = {}

            def emit_pv(pc, pt_, ppts):
                if pt_ == 0:
                    ot_a = ps_ot.tile([HD + 1, QC], fp32, tag="ot")
                    ot_b = ps_ot.tile([HD + 1, QC], fp32, tag="ot")
                    ots[pc] = (ot_a, ot_b)
                for o, pp in zip(ots[pc], ppts):
                    for u in range(QC // MM_N):
                        nc.tensor.matmul(
                            o[:, bass.ts(u, MM_N)], v_sb[:, pt_, :],
                            pp[:, bass.ts(u, MM_N)],
                            start=(pt_ == 0), stop=(pt_ == NKT - 1),
                        )

            otcps = {}

            def emit_otcp(pc):
                # Free the PSUM accumulators fast (one DVE copy each) so the
                # next chunk's PV matmuls aren't blocked behind the slow
                # normalization chain; the divide happens lazily off ot_cp.
                cp_a = otcp_pool.tile([HD + 1, QC], fp32, tag="otcp")
                cp_b = otcp_pool.tile([HD + 1, QC], fp32, tag="otcp")
                nc.vector.tensor_copy(cp_a, ots[pc][0])
                nc.scalar.activation(  # ACT is idle here; halves the wait
                    cp_b, ots[pc][1], mybir.ActivationFunctionType.Copy
                )
                otcps[pc] = (cp_a, cp_b)
                del ots[pc]

            def emit_norm_piece(pc, h, u, use_act=False):
                # ot rows 0..63 / row 64 (denominators): reciprocal, round to
                # bf16, broadcast across the 64 hd partitions with a rank-1
                # PE matmul (ones column), multiply on DVE. The DVE IEEE
                # reciprocal costs ~3.3us per 512 elems, so in steady state it
                # runs off the critical path; in the epilogue (use_act=True)
                # the idle Scalar engine computes 1/d = exp(-ln d) instead.
                o = otcps[pc][h]
                usl = bass.ts(u, MM_N)
                rsb = nrm_pool.tile([1, MM_N], bf16, tag="rsb")
                if use_act:
                    rs = nrm_pool.tile([1, MM_N], fp32, tag="rs")
                    nc.scalar.activation(
                        rs, o[HD:HD + 1, usl],
                        mybir.ActivationFunctionType.Ln,
                    )
                    nc.scalar.activation(rsb, rs, exp, scale=-1.0)
                else:
                    rs = nrm_pool.tile([1, MM_N], fp32, tag="rs")
                    nc.vector.reciprocal(rs, o[HD:HD + 1, usl])
                    nc.vector.tensor_copy(rsb, rs)
                bc_ps = ps_st.tile([HD, MM_N], fp32, tag="st")
                nc.tensor.matmul(bc_ps, ones_col, rsb, start=True, stop=True)
                nc.vector.tensor_mul(
                    ot_full[h * HD:(h + 1) * HD,
                            pc * QC + u * MM_N:pc * QC + (u + 1) * MM_N],
                    o[0:HD, usl], bc_ps,
                )

            def outproj_piece(jq):
                for u2 in range(D // MM_N):
                    yp = ps_st.tile([128, MM_N], fp32, tag="st")
                    nc.tensor.matmul(
                        yp, ot_full[:, bass.ts(jq, 128)],
                        ow_sb[:, bass.ts(u2, MM_N)],
                        start=True, stop=True,
                    )
                    ysb = ysb_pool.tile([128, MM_N], bf16, tag="ysb")
                    nc.vector.tensor_copy(ysb, yp)
                    nc.sync.dma_start(
                        out=y[:, :][bass.ts(jq, 128), bass.ts(u2, MM_N)],
                        in_=ysb,
                    )

            # ---- prologue: all KV slices (xt-DMA paced), Q proj chunk 0.
            # Keeping DMA-dependent work out of the attention loop avoids
            # wait-queue pileups that collapse the PE into its mid p-state.
            for j in range(4):
                kv_slice(j)
            proj_slice(qt_sb, qw_sb, qb_sb, 0)
            proj_slice(qt_sb, qw_sb, qb_sb, 1)

            # ---- main software-pipelined loop ----
            # norm pieces (h, u) for the previous chunk run at steps 5-17,
            # spaced so each DVE reciprocal (~3.3us) completes before its
            # PE broadcast matmul is enqueued (no head-of-line blocking);
            # outproj pieces follow once their ot_full half is normalized.
            NORM_STEPS = (5, 9, 13, 17)
            OUTPROJ_STEPS = (11, 13, 15, 17, 19, 21, 23, 25)
            pending_outproj = []
            pending_norm = []
            prev = None
            for s_ in range(NQC * NKT):
                c, t = divmod(s_, NKT)
                pts = emit_scores(c, t)
                if prev is not None:
                    (pc, pt_), ppts = prev
                    emit_pv(pc, pt_, ppts)
                    if pt_ == NKT - 1:
                        emit_otcp(pc)
                        pending_norm = [
                            (pc, h, u) for u in range(2) for h in range(2)
                        ]
                        pending_outproj.extend(
                            range(pc * (QC // 128), (pc + 1) * (QC // 128))
                        )
                # ---- PE filler work (keeps the systolic array saturated) ----
                if pending_norm and t in NORM_STEPS:
                    emit_norm_piece(*pending_norm.pop(0))
                if c == 0 and t in (6, 10, 14, 18):
                    kv_slice(4 + (t - 6) // 4)
                if c + 1 < NQC:
                    if t == (24 if c == 0 else 6):
                        proj_slice(qt_sb, qw_sb, qb_sb, 2 * (c + 1))
                    elif t == (28 if c == 0 else 22):
                        proj_slice(qt_sb, qw_sb, qb_sb, 2 * (c + 1) + 1)
                if pending_outproj and t in OUTPROJ_STEPS:
                    outproj_piece(pending_outproj.pop(0))
                prev = ((c, t), pts)

            # ---- epilogue: last chunk's PV tail, norm via idle ScalarE,
            # outproj pieces interleaved as their ot_full halves finish ----
            (pc, pt_), ppts = prev
            emit_pv(pc, pt_, ppts)
            emit_otcp(pc)
            jq0 = pc * (QC // 128)
            for u in range(2):
                # h1 reciprocal on DVE and h0 via ScalarE ln/exp in parallel
                emit_norm_piece(pc, 1, u, use_act=False)
                emit_norm_piece(pc, 0, u, use_act=True)
                for jq in range(jq0 + 4 * u, jq0 + 4 * u + 4):
                    outproj_piece(jq)
            for jq in pending_outproj:
                outproj_piece(jq)
    _split_multi_waits(nc)
    return nc


def _split_multi_waits(nc):
    """This toolchain's walrus accepts at most one sync-wait per datapath
    instruction; move extra waits onto same-engine NoOps placed just before."""
    k = 0
    for f in nc.m.functions:
        for blk in f.blocks:
            out = []
            for inst in blk.instructions:
                si = getattr(inst, "sync_info", None)
                ow_ = list(si.on_wait) if (si and si.on_wait) else []
                if len(ow_) > 1:
                    for w in ow_[:-1]:
                        k += 1
                        nop = bass_rust.InstNoOp(
                            name=f"I-wsplit-{k}", ins=[], outs=[]
                        )
                        nop.engine = inst.engine
                        nop.sync_info = mybir.SyncInfo(
                            on_wait=[w], on_update=[]
                        )
                        out.append(nop)
                    inst.sync_info = mybir.SyncInfo(
                        on_wait=[ow_[-1]], on_update=list(si.on_update or [])
                    )
                out.append(inst)
            blk.instructions = out


def _prep_inputs(X, q_w, q_b, k_w, k_b, v_w, v_b, o_w):
    Xt = np.ascontiguousarray(X.reshape(S, D).T).astype(BF16)
    in_maps = []
    for c in range(NCORES):
        kv = c // (NCORES // KVH)
        qs = slice(c * DQ, (c + 1) * DQ)
        ks = slice(kv * HD, (kv + 1) * HD)
        in_maps.append({
            "xt": Xt,
            "qw": np.ascontiguousarray(q_w[:, qs]).astype(BF16),
            "kvw": np.ascontiguousarray(
                np.concatenate([k_w[:, ks], v_w[:, ks]], axis=1)).astype(BF16),
            "ow": np.ascontiguousarray(o_w[qs, :]).astype(BF16),
            "qb": np.ascontiguousarray(q_b[qs]).reshape(DQ, 1).astype(
                np.float32),
            "kvb": np.ascontiguousarray(
                np.concatenate([k_b[ks], v_b[ks]])).reshape(DKV, 1).astype(
                np.float32),
        })
    return in_maps


def kernel(X, q_w, q_b, k_w, k_b, v_w, v_b, o_w, o_b, **run_kwargs):
    global _COMPILED
    if _COMPILED is None:
        _COMPILED = build_bass()
    in_maps = _prep_inputs(X, q_w, q_b, k_w, k_b, v_w, v_b, o_w)
    res = run_bass_kernel_spmd(
        _COMPILED, in_maps, list(range(NCORES)), **run_kwargs
    )
    parts = [r["y"] for r in res.results]
    out = parts[0].astype(np.float32)
    for p in parts[1:]:
        out = out + p
    out = out + o_b.astype(np.float32)[None, :]
    if run_kwargs:
        return out.reshape(B, S, D), res
    return out.reshape(B, S, D)



# revision 6
# speedup vs baseline: 1.1094x; 1.1094x over previous
"""GQA attention kernel for Trainium2, sharded over 8 NeuronCores.

Problem: X (1, 4096, 1024), H=16 q-heads, KVH=4 kv-heads, head_dim=64.
Sharding: 2 q-heads + their shared kv-head per core (tensor parallel over H).
Each core computes q/k/v projections for its heads, fused attention, and the
per-head slice of the output projection -> partial (4096, 1024), summed on
host.

v3: the kernel is ACT(exp)-bound -- softmax exp is 33.5M elements/core at
1 elem/lane/cycle @1.2GHz (~272us incl. per-instruction overhead). Design
pins ACT at ~100% busy and fits all PE work underneath it:
  - 512-q steps: both heads' score matmuls write one 2-bank PSUM tile as a
    row-tiled T0/T8 pair (K=64 -> 64x128 PE tiles, concurrent: the two MMs
    occupy disjoint halves of the systolic array), and ONE fused ACTIVATE
    exponentiates both heads' scores ([128,1024], minimizing the ~250-cycle
    per-ACTIVATE overhead).
  - PV keeps the V_aug ones-row trick (M=65) for free softmax denominators.
  - The normalization moved AFTER the output projection: outproj runs as
    per-head K=64 row-tiled T0/T8 pairs, and y = Ya*(1/da) + Yb*(1/db) on
    DVE with per-partition scalars. Denominators are PE-transposed into
    partition-major [128,4] blocks so the DVE reciprocal runs 128 lanes wide
    (the v2 [1,512] reciprocals burned 3.3us each on one lane).
  - q/kv projections and V transposes are spread as PE filler through the
    step windows (JIT), PSUM: 4 banks scores (double-buffered) + 2 banks PV
    accumulators + 2 banks shared transients = 8.

Layouts on device (per core):
  xt   : X^T            (1024 D, 4096 S)  bf16   (host pre-transposed)
  qt   : Q^T            (128 = 2 heads x 64 d, 4096 q) bf16
  kvt  : [K^T; V^T]     (128 = 64 k-d + 64 v-d, 4096 s) bf16
  kt2  : K^T duplicated into both partition halves
  v    : V natural+ones (128 s-tile, 65) x 32 tiles bf16 (col 64 == 1.0)
  st   : scores^T pair  (128 k, 2x512 q) f32 PSUM  = Kt.T @ Qt  (T0 | T8)
  pt   : exp(st/8)      (128 k, 1024) bf16 SBUF (one fused ACTIVATE)
  ot   : V_aug.T @ Pt   (65, 512) f32 PSUM per head; row 64 = denominators
  otf  : unnormalized O^T (128, 4096) bf16
  y    : partial output (4096, 1024) bf16 = Ya/da + Yb/db  per 128-q tile
"""

import sys

import numpy as np

try:
    import concourse.bass as bass
except ImportError:  # grading env may not have concourse on sys.path
    for p in ("/opt/trn_rl_repo", "/root/.axon_site/_ro/trn_rl_repo"):
        if p not in sys.path:
            sys.path.append(p)
    import concourse.bass as bass

import bass_rust
import ml_dtypes
from concourse import mybir
from concourse.bass_utils import run_bass_kernel_spmd
from concourse.masks import make_identity
from concourse.tile import TileContext

BF16 = ml_dtypes.bfloat16

B, S, D = 1, 4096, 1024
H, KVH, HD = 16, 4, 64
NCORES = 8
HPC = H // NCORES          # 2 q heads per core
DQ = HPC * HD              # 128 projected q dims per core
DKV = 2 * HD               # 128 = k head + v head dims
QC = 512                   # attention q-chunk per step
KT = 128                   # k tile (seq positions per score tile)
NKT = S // KT              # 32
NCH = S // QC              # 8 chunks
NDC = D // 128             # 8 contraction chunks for projections
MM_N = 512                 # max matmul free dim (one PSUM bank, f32)

_COMPILED = None


def build_bass():
    nc = bass.Bass()
    fp32 = mybir.dt.float32
    bf16 = mybir.dt.bfloat16
    exp = mybir.ActivationFunctionType.Exp
    MULT = mybir.AluOpType.mult
    ADD = mybir.AluOpType.add

    xt = nc.declare_dram_parameter("xt", [D, S], bf16, isOutput=False)
    qw = nc.declare_dram_parameter("qw", [D, DQ], bf16, isOutput=False)
    kvw = nc.declare_dram_parameter("kvw", [D, DKV], bf16, isOutput=False)
    ow = nc.declare_dram_parameter("ow", [DQ, D], bf16, isOutput=False)
    qb = nc.declare_dram_parameter("qb", [DQ, 1], fp32, isOutput=False)
    kvb = nc.declare_dram_parameter("kvb", [DKV, 1], fp32, isOutput=False)
    y = nc.declare_dram_parameter("y", [S, D], bf16, isOutput=True)

    with TileContext(nc) as tc:
        with (
            tc.tile_pool(name="singles", bufs=1) as singles,
            tc.tile_pool(name="pt_pool", bufs=3) as pt_pool,
            tc.tile_pool(name="ytmp", bufs=2) as ytmp_pool,
            tc.tile_pool(name="ysb", bufs=4) as ysb_pool,
            tc.tile_pool(name="ps_st", bufs=2, space="PSUM") as ps_st,
            tc.tile_pool(name="ps_ot", bufs=2, space="PSUM") as ps_ot,
            tc.tile_pool(name="ps_tr", bufs=1, space="PSUM") as ps_tr,
        ):
            # ---- constants / weights ----
            ident = singles.tile([128, 128], bf16)
            make_identity(nc, ident)
            identf = singles.tile([2, 2], fp32)
            make_identity(nc, identf)

            # DMA order: q/kv weights + first xt blocks first (prologue
            # needs them); ow last (first used ~50us in).
            xt_sb = singles.tile([128, NDC, S], bf16)
            xt_re = xt[:, :].rearrange("(c p) s -> p c s", p=128)
            kvw_sb = singles.tile([128, NDC, DKV], bf16)
            kvw_re = kvw[:, :].rearrange("(c p) m -> p c m", p=128)
            qw_sb = singles.tile([128, NDC, DQ], bf16)
            qw_re = qw[:, :].rearrange("(c p) m -> p c m", p=128)
            for half in range(2):
                nc.sync.dma_start(
                    out=kvw_sb[:, bass.ts(half, 4), :],
                    in_=kvw_re[:, bass.ts(half, 4), :],
                )
                nc.sync.dma_start(
                    out=qw_sb[:, bass.ts(half, 4), :],
                    in_=qw_re[:, bass.ts(half, 4), :],
                )
            qb_sb = singles.tile([DQ, 1], fp32)
            nc.sync.dma_start(out=qb_sb, in_=qb[:, :])
            kvb_sb = singles.tile([DKV, 1], fp32)
            nc.sync.dma_start(out=kvb_sb, in_=kvb[:, :])
            for j in range(4):
                for ch in range(2):
                    nc.sync.dma_start(
                        out=xt_sb[:, bass.ts(ch, 4), bass.ts(j, 1024)],
                        in_=xt_re[:, bass.ts(ch, 4), bass.ts(j, 1024)],
                    )
            ow_sb = singles.tile([DQ, D], bf16)
            nc.sync.dma_start(out=ow_sb, in_=ow[:, :])

            qt_sb = singles.tile([DQ, S], bf16)
            kvt_sb = singles.tile([DKV, S], bf16)
            kt2_sb = singles.tile([DKV, S], bf16)
            v_sb = singles.tile([128, NKT, HD + 1], bf16)
            nc.vector.memset(v_sb, 1.0)
            ot_full = singles.tile([DQ, S], bf16)
            den_sb = singles.tile([1, 2, NCH, QC], fp32)
            rsb_sb = singles.tile([128, NCH, 8], fp32)

            # ---------------- helpers ----------------
            def proj_mms(state, w_sb, j, k):
                # two accumulating c-chunk matmuls of a 512-col projection
                if k == 0:
                    state["ps"] = ps_tr.tile([128, 1024], fp32, tag="tr", name="projps")
                ps = state["ps"]
                for c2 in (2 * k, 2 * k + 1):
                    nc.tensor.matmul(
                        ps[:, 0:MM_N], w_sb[:, c2, :],
                        xt_sb[:, c2, bass.ts(j, MM_N)],
                        start=(c2 == 0), stop=(c2 == NDC - 1),
                    )

            def proj_fin(state, dst, b_sb, j):
                nc.vector.tensor_scalar_add(
                    dst[:, bass.ts(j, MM_N)], state.pop("ps")[:, 0:MM_N],
                    b_sb[:, 0:1],
                )

            def kt2_dup(j):
                nc.sync.dma_start(
                    out=kt2_sb[0:HD, bass.ts(j, MM_N)],
                    in_=kvt_sb[0:HD, bass.ts(j, MM_N)],
                )
                nc.sync.dma_start(
                    out=kt2_sb[HD:DKV, bass.ts(j, MM_N)],
                    in_=kvt_sb[0:HD, bass.ts(j, MM_N)],
                )

            def v_transpose(tt):
                pvt = ps_tr.tile([128, HD], bf16, tag="tr", name="pvt")
                nc.tensor.transpose(
                    pvt, kvt_sb[HD:DKV, bass.ts(tt, KT)],
                    ident[HD:DKV, HD:DKV],
                )
                nc.vector.tensor_copy(v_sb[:, tt, 0:HD], pvt)

            def kv_slice_full(j):
                st_ = {}
                for k in range(4):
                    proj_mms(st_, kvw_sb, j, k)
                proj_fin(st_, kvt_sb, kvb_sb, j)
                kt2_dup(j)
                for tt in range(4 * j, 4 * j + 4):
                    v_transpose(tt)

            def q_slice_full(j):
                st_ = {}
                for k in range(4):
                    proj_mms(st_, qw_sb, j, k)
                proj_fin(st_, qt_sb, qb_sb, j)

            # ---- attention step pieces ----
            ots = {}

            def emit_scores(c, t):
                st = ps_st.tile([128, 2 * QC], fp32, tag="st", name="st")
                nc.tensor.matmul(
                    st[:, 0:QC],
                    kt2_sb[0:HD, bass.ts(t, KT)],
                    qt_sb[0:HD, c * QC:(c + 1) * QC],
                    start=True, stop=True,
                )
                nc.tensor.matmul(
                    st[:, QC:2 * QC],
                    kt2_sb[HD:DKV, bass.ts(t, KT)],
                    qt_sb[HD:DKV, c * QC:(c + 1) * QC],
                    start=True, stop=True,
                )
                pt = pt_pool.tile([128, 2 * QC], bf16, tag="pt", name="pt")
                nc.scalar.activation(pt, st, exp, scale=1.0 / 8.0)
                return pt

            def emit_pv(pc, pt_, ptile):
                if pt_ == 0:
                    ot_a = ps_ot.tile([HD + 1, QC], fp32, tag="ot", name="ot_a")
                    ot_b = ps_ot.tile([HD + 1, QC], fp32, tag="ot", name="ot_b")
                    ots[pc] = (ot_a, ot_b)
                ot_a, ot_b = ots[pc]
                nc.tensor.matmul(
                    ot_a, v_sb[:, pt_, :], ptile[:, 0:QC],
                    start=(pt_ == 0), stop=(pt_ == NKT - 1),
                )
                nc.tensor.matmul(
                    ot_b, v_sb[:, pt_, :], ptile[:, QC:2 * QC],
                    start=(pt_ == 0), stop=(pt_ == NKT - 1),
                )

            def emit_otcp(pc):
                # unnormalized O^T -> SBUF; denominators -> den_sb staging
                ot_a, ot_b = ots.pop(pc)
                nc.vector.tensor_copy(
                    ot_full[0:HD, bass.ts(pc, QC)], ot_a[0:HD, :]
                )
                nc.vector.tensor_copy(
                    ot_full[HD:DKV, bass.ts(pc, QC)], ot_b[0:HD, :]
                )
                nc.vector.tensor_copy(
                    den_sb[0:1, 0, pc, :], ot_a[HD:HD + 1, :]
                )
                nc.vector.tensor_copy(
                    den_sb[0:1, 1, pc, :], ot_b[HD:HD + 1, :]
                )

            def emit_dtr(pc):
                # transpose denominators to partition-major, 128-lane recip:
                # dps col 2u+h = denominators for head h, q block u
                dps = ps_tr.tile([128, 8], fp32, tag="tr", name="dps")
                for u in range(4):
                    for h in range(2):
                        nc.tensor.transpose(
                            dps[:, 2 * u + h:2 * u + h + 1],
                            den_sb[0:1, h, pc, bass.ts(u, 128)],
                            identf[0:1, 0:1],
                        )
                nc.vector.reciprocal(rsb_sb[:, pc, :], dps)

            def emit_y(pc, jq, u2):
                # outproj for q rows [pc*512+jq*128, +128), d cols u2*512:
                # per-head K=64 row-tiled pair, then normalize-and-sum on DVE
                yp = ps_tr.tile([128, 1024], fp32, tag="tr", name="yp")
                qcol = pc * QC + jq * KT
                nc.tensor.matmul(
                    yp[:, 0:MM_N], ot_full[0:HD, qcol:qcol + KT],
                    ow_sb[0:HD, bass.ts(u2, MM_N)],
                    start=True, stop=True,
                )
                nc.tensor.matmul(
                    yp[:, MM_N:2 * MM_N], ot_full[HD:DKV, qcol:qcol + KT],
                    ow_sb[HD:DKV, bass.ts(u2, MM_N)],
                    start=True, stop=True,
                )
                tmp = ytmp_pool.tile([128, MM_N], fp32, tag="yt", name="ytmp")
                nc.vector.tensor_scalar_mul(
                    tmp, yp[:, MM_N:2 * MM_N],
                    rsb_sb[:, pc, 2 * jq + 1:2 * jq + 2],
                )
                ysb = ysb_pool.tile([128, MM_N], bf16, tag="ysb", name="ysb")
                nc.vector.scalar_tensor_tensor(
                    ysb, yp[:, 0:MM_N], rsb_sb[:, pc, 2 * jq:2 * jq + 1],
                    tmp, MULT, ADD,
                )
                nc.sync.dma_start(
                    out=y[:, :][qcol:qcol + KT, bass.ts(u2, MM_N)], in_=ysb,
                )

            # ---------------- filler schedule ----------------
            # fillers[(c, t)] -> list of thunks, run after scores/PV of that
            # step. Chunk 0: kv slices 4-7 + q slice 2. Chunks c>=1: denom
            # transposes (w0), Y events (w2..16 even), q slice c+2 (w20..24).
            fillers = {}

            def add_filler(c, t, fn):
                fillers.setdefault((c, t), []).append(fn)

            for s_i, w0 in ((4, 0), (5, 6), (6, 12), (7, 18)):
                st_ = {}
                for k in range(4):
                    add_filler(
                        0, w0 + k,
                        lambda st_=st_, j=s_i, k=k: proj_mms(st_, kvw_sb, j, k),
                    )
                add_filler(
                    0, w0 + 4,
                    lambda st_=st_, j=s_i: (
                        proj_fin(st_, kvt_sb, kvb_sb, j), kt2_dup(j),
                        v_transpose(4 * j), v_transpose(4 * j + 1),
                    ),
                )
                add_filler(
                    0, w0 + 5,
                    lambda j=s_i: (
                        v_transpose(4 * j + 2), v_transpose(4 * j + 3),
                    ),
                )
            qst = {}
            for k in range(4):
                add_filler(
                    0, 26 + k,
                    lambda k=k: proj_mms(qst, qw_sb, 2, k),
                )
            add_filler(0, 30, lambda: proj_fin(qst, qt_sb, qb_sb, 2))

            for c in range(1, NCH):
                pc = c - 1
                add_filler(c, 0, lambda pc=pc: emit_dtr(pc))
                for i in range(8):
                    add_filler(
                        c, 2 + 2 * i,
                        lambda pc=pc, jq=i // 2, u2=i % 2: emit_y(pc, jq, u2),
                    )
                if c + 2 < NCH:
                    qst_c = {}
                    for k in range(4):
                        add_filler(
                            c, 20 + k,
                            lambda d=qst_c, j=c + 2, k=k: proj_mms(d, qw_sb, j, k),
                        )
                    add_filler(
                        c, 24,
                        lambda d=qst_c, j=c + 2: proj_fin(d, qt_sb, qb_sb, j),
                    )

            # ---------------- prologue ----------------
            for j in range(4):
                kv_slice_full(j)
            q_slice_full(0)
            q_slice_full(1)

            # ---------------- main loop ----------------
            prev = None
            for step in range(NCH * NKT):
                c, t = divmod(step, NKT)
                pt = emit_scores(c, t)
                if prev is not None:
                    (pc, pt_), pptile = prev
                    emit_pv(pc, pt_, pptile)
                    if pt_ == NKT - 1:
                        emit_otcp(pc)
                for fn in fillers.get((c, t), ()):
                    fn()
                prev = ((c, t), pt)

            # ---------------- epilogue ----------------
            (pc, pt_), pptile = prev
            emit_pv(pc, pt_, pptile)
            emit_otcp(pc)
            emit_dtr(pc)
            for i in range(8):
                emit_y(pc, i // 2, i % 2)
    _split_multi_waits(nc)
    return nc


def _split_multi_waits(nc):
    """This toolchain's walrus accepts at most one sync-wait per datapath
    instruction; move extra waits onto same-engine NoOps placed just before."""
    k = 0
    for f in nc.m.functions:
        for blk in f.blocks:
            out = []
            for inst in blk.instructions:
                si = getattr(inst, "sync_info", None)
                ow_ = list(si.on_wait) if (si and si.on_wait) else []
                if len(ow_) > 1:
                    for w in ow_[:-1]:
                        k += 1
                        nop = bass_rust.InstNoOp(
                            name=f"I-wsplit-{k}", ins=[], outs=[]
                        )
                        nop.engine = inst.engine
                        nop.sync_info = mybir.SyncInfo(
                            on_wait=[w], on_update=[]
                        )
                        out.append(nop)
                    inst.sync_info = mybir.SyncInfo(
                        on_wait=[ow_[-1]], on_update=list(si.on_update or [])
                    )
                out.append(inst)
            blk.instructions = out


def _prep_inputs(X, q_w, q_b, k_w, k_b, v_w, v_b, o_w):
    Xt = np.ascontiguousarray(X.reshape(S, D).T).astype(BF16)
    in_maps = []
    for c in range(NCORES):
        kv = c // (NCORES // KVH)
        qs = slice(c * DQ, (c + 1) * DQ)
        ks = slice(kv * HD, (kv + 1) * HD)
        in_maps.append({
            "xt": Xt,
            "qw": np.ascontiguousarray(q_w[:, qs]).astype(BF16),
            "kvw": np.ascontiguousarray(
                np.concatenate([k_w[:, ks], v_w[:, ks]], axis=1)).astype(BF16),
            "ow": np.ascontiguousarray(o_w[qs, :]).astype(BF16),
            "qb": np.ascontiguousarray(q_b[qs]).reshape(DQ, 1).astype(
                np.float32),
            "kvb": np.ascontiguousarray(
                np.concatenate([k_b[ks], v_b[ks]])).reshape(DKV, 1).astype(
                np.float32),
        })
    return in_maps


def kernel(X, q_w, q_b, k_w, k_b, v_w, v_b, o_w, o_b, **run_kwargs):
    global _COMPILED
    if _COMPILED is None:
        _COMPILED = build_bass()
    in_maps = _prep_inputs(X, q_w, q_b, k_w, k_b, v_w, v_b, o_w)
    res = run_bass_kernel_spmd(
        _COMPILED, in_maps, list(range(NCORES)), **run_kwargs
    )
    parts = [r["y"] for r in res.results]
    out = parts[0].astype(np.float32)
    for p in parts[1:]:
        out = out + p
    out = out + o_b.astype(np.float32)[None, :]
    if run_kwargs:
        return out.reshape(B, S, D), res
    return out.reshape(B, S, D)


# revision 9
# speedup vs baseline: 1.2055x; 1.0866x over previous
"""GQA attention kernel for Trainium2, sharded over 8 NeuronCores.

Problem: X (1, 4096, 1024), H=16 q-heads, KVH=4 kv-heads, head_dim=64.
Sharding: 2 q-heads + their shared kv-head per core (tensor parallel over H).
Each core computes q/k/v projections for its heads, fused attention, and the
per-head slice of the output projection -> partial (4096, 1024), summed on
host.

v3: the kernel is ACT(exp)-bound -- softmax exp is 33.5M elements/core at
1 elem/lane/cycle @1.2GHz (~272us incl. per-instruction overhead). Design
pins ACT at ~100% busy and fits all PE work underneath it:
  - 512-q steps: both heads' score matmuls write one 2-bank PSUM tile as a
    row-tiled T0/T8 pair (K=64 -> 64x128 PE tiles, concurrent: the two MMs
    occupy disjoint halves of the systolic array), and ONE fused ACTIVATE
    exponentiates both heads' scores ([128,1024], minimizing the ~250-cycle
    per-ACTIVATE overhead).
  - PV keeps the V_aug ones-row trick (M=65) for free softmax denominators.
  - The normalization moved AFTER the output projection: outproj runs as
    per-head K=64 row-tiled T0/T8 pairs, and y = Ya*(1/da) + Yb*(1/db) on
    DVE with per-partition scalars. Denominators are PE-transposed into
    partition-major [128,4] blocks so the DVE reciprocal runs 128 lanes wide
    (the v2 [1,512] reciprocals burned 3.3us each on one lane).
  - q/kv projections and V transposes are spread as PE filler through the
    step windows (JIT), PSUM: 4 banks scores (double-buffered) + 2 banks PV
    accumulators + 2 banks shared transients = 8.

Layouts on device (per core):
  xt   : X^T            (1024 D, 4096 S)  bf16   (host pre-transposed)
  qt   : Q^T            (128 = 2 heads x 64 d, 4096 q) bf16
  kvt  : [K^T; V^T]     (128 = 64 k-d + 64 v-d, 4096 s) bf16
  kt2  : K^T duplicated into both partition halves
  v    : V natural+ones (128 s-tile, 65) x 32 tiles bf16 (col 64 == 1.0)
  st   : scores^T pair  (128 k, 2x512 q) f32 PSUM  = Kt.T @ Qt  (T0 | T8)
  pt   : exp(st/8)      (128 k, 1024) bf16 SBUF (one fused ACTIVATE)
  ot   : V_aug.T @ Pt   (65, 512) f32 PSUM per head; row 64 = denominators
  otf  : unnormalized O^T (128, 4096) bf16
  y    : partial output (4096, 1024) bf16 = Ya/da + Yb/db  per 128-q tile
"""

import sys

import numpy as np

try:
    import concourse.bass as bass
except ImportError:  # grading env may not have concourse on sys.path
    for p in ("/opt/trn_rl_repo", "/root/.axon_site/_ro/trn_rl_repo"):
        if p not in sys.path:
            sys.path.append(p)
    import concourse.bass as bass

import bass_rust
import ml_dtypes
from concourse import mybir
from concourse.bass_utils import run_bass_kernel_spmd
from concourse.masks import make_identity
from concourse.tile import TileContext

BF16 = ml_dtypes.bfloat16

B, S, D = 1, 4096, 1024
H, KVH, HD = 16, 4, 64
NCORES = 8
HPC = H // NCORES          # 2 q heads per core
DQ = HPC * HD              # 128 projected q dims per core
DKV = 2 * HD               # 128 = k head + v head dims
QC = 512                   # attention q-chunk per step
KT = 128                   # k tile (seq positions per score tile)
NKT = S // KT              # 32
NCH = S // QC              # 8 chunks
NDC = D // 128             # 8 contraction chunks for projections
MM_N = 512                 # max matmul free dim (one PSUM bank, f32)

_COMPILED = None


def build_bass():
    nc = bass.Bass()
    fp32 = mybir.dt.float32
    bf16 = mybir.dt.bfloat16
    exp = mybir.ActivationFunctionType.Exp
    MULT = mybir.AluOpType.mult
    ADD = mybir.AluOpType.add

    xt = nc.declare_dram_parameter("xt", [D, S], bf16, isOutput=False)
    qw = nc.declare_dram_parameter("qw", [D, DQ], bf16, isOutput=False)
    kvw = nc.declare_dram_parameter("kvw", [D, DKV], bf16, isOutput=False)
    ow = nc.declare_dram_parameter("ow", [DQ, D], bf16, isOutput=False)
    qb = nc.declare_dram_parameter("qb", [DQ, 1], fp32, isOutput=False)
    kvb = nc.declare_dram_parameter("kvb", [DKV, 1], fp32, isOutput=False)
    y = nc.declare_dram_parameter("y", [S, D], bf16, isOutput=True)

    with TileContext(nc) as tc:
        with (
            tc.tile_pool(name="singles", bufs=1) as singles,
            tc.tile_pool(name="pt_pool", bufs=3) as pt_pool,
            tc.tile_pool(name="ytmp", bufs=2) as ytmp_pool,
            tc.tile_pool(name="ysb", bufs=4) as ysb_pool,
            tc.tile_pool(name="ps_st", bufs=2, space="PSUM") as ps_st,
            tc.tile_pool(name="ps_ot", bufs=2, space="PSUM") as ps_ot,
            tc.tile_pool(name="ps_tr", bufs=2, space="PSUM") as ps_tr,
        ):
            # ---- constants / weights ----
            ident = singles.tile([128, 128], bf16)
            make_identity(nc, ident)
            identf = singles.tile([2, 2], fp32)
            make_identity(nc, identf)

            # DMA order: q/kv weights + first xt blocks first (prologue
            # needs them); ow last (first used ~50us in).
            xt_sb = singles.tile([128, NDC, S], bf16)
            xt_re = xt[:, :].rearrange("(c p) s -> p c s", p=128)
            kvw_sb = singles.tile([128, NDC, DKV], bf16)
            kvw_re = kvw[:, :].rearrange("(c p) m -> p c m", p=128)
            qw_sb = singles.tile([128, NDC, DQ], bf16)
            qw_re = qw[:, :].rearrange("(c p) m -> p c m", p=128)
            for half in range(2):
                nc.sync.dma_start(
                    out=kvw_sb[:, bass.ts(half, 4), :],
                    in_=kvw_re[:, bass.ts(half, 4), :],
                )
                nc.sync.dma_start(
                    out=qw_sb[:, bass.ts(half, 4), :],
                    in_=qw_re[:, bass.ts(half, 4), :],
                )
            qb_sb = singles.tile([DQ, 1], fp32)
            nc.sync.dma_start(out=qb_sb, in_=qb[:, :])
            kvb_sb = singles.tile([DKV, 1], fp32)
            nc.sync.dma_start(out=kvb_sb, in_=kvb[:, :])
            for j in range(4):
                for ch in range(2):
                    nc.sync.dma_start(
                        out=xt_sb[:, bass.ts(ch, 4), bass.ts(j, 1024)],
                        in_=xt_re[:, bass.ts(ch, 4), bass.ts(j, 1024)],
                    )
            ow_sb = singles.tile([DQ, D], bf16)
            nc.sync.dma_start(out=ow_sb, in_=ow[:, :])

            qt_sb = singles.tile([DQ, S], bf16)
            kvt_sb = singles.tile([DKV, S], bf16)
            kt2_sb = singles.tile([DKV, S], bf16)
            v_sb = singles.tile([128, NKT, HD + 1], bf16)
            nc.vector.memset(v_sb, 1.0)
            ot_full = singles.tile([DQ, S], bf16)
            den_sb = singles.tile([1, 2, NCH, QC], fp32)
            rsb_sb = singles.tile([128, NCH, 8], fp32)

            # ---------------- helpers ----------------
            def proj_mms(state, w_sb, j, k):
                # two accumulating c-chunk matmuls of a 512-col projection
                if k == 0:
                    state["ps"] = ps_tr.tile(
                        [128, MM_N], fp32, tag="tr", name="projps"
                    )
                ps = state["ps"]
                for c2 in (2 * k, 2 * k + 1):
                    nc.tensor.matmul(
                        ps, w_sb[:, c2, :],
                        xt_sb[:, c2, bass.ts(j, MM_N)],
                        start=(c2 == 0), stop=(c2 == NDC - 1),
                    )

            def proj_fin(state, dst, b_sb, j):
                nc.vector.tensor_scalar_add(
                    dst[:, bass.ts(j, MM_N)], state.pop("ps"),
                    b_sb[:, 0:1],
                )

            def kt2_dup(j):
                nc.sync.dma_start(
                    out=kt2_sb[0:HD, bass.ts(j, MM_N)],
                    in_=kvt_sb[0:HD, bass.ts(j, MM_N)],
                )
                nc.sync.dma_start(
                    out=kt2_sb[HD:DKV, bass.ts(j, MM_N)],
                    in_=kvt_sb[0:HD, bass.ts(j, MM_N)],
                )

            def v_transpose(tt):
                pvt = ps_tr.tile([128, HD], bf16, tag="tr", name="pvt")
                nc.tensor.transpose(
                    pvt, kvt_sb[HD:DKV, bass.ts(tt, KT)],
                    ident[HD:DKV, HD:DKV],
                )
                nc.vector.tensor_copy(v_sb[:, tt, 0:HD], pvt)

            def kv_slice_full(j):
                st_ = {}
                for k in range(4):
                    proj_mms(st_, kvw_sb, j, k)
                proj_fin(st_, kvt_sb, kvb_sb, j)
                kt2_dup(j)
                for tt in range(4 * j, 4 * j + 4):
                    v_transpose(tt)

            def q_slice_full(j):
                st_ = {}
                for k in range(4):
                    proj_mms(st_, qw_sb, j, k)
                proj_fin(st_, qt_sb, qb_sb, j)

            # ---- attention step pieces ----
            ots = {}

            def emit_scores(c, t):
                st = ps_st.tile([128, 2 * QC], fp32, tag="st", name="st")
                nc.tensor.matmul(
                    st[:, 0:QC],
                    kt2_sb[0:HD, bass.ts(t, KT)],
                    qt_sb[0:HD, c * QC:(c + 1) * QC],
                    start=True, stop=True,
                )
                nc.tensor.matmul(
                    st[:, QC:2 * QC],
                    kt2_sb[HD:DKV, bass.ts(t, KT)],
                    qt_sb[HD:DKV, c * QC:(c + 1) * QC],
                    start=True, stop=True,
                )
                pt = pt_pool.tile([128, 2 * QC], bf16, tag="pt", name="pt")
                nc.scalar.activation(pt, st, exp, scale=1.0 / 8.0)
                return pt

            def emit_pv(pc, pt_, ptile):
                if pt_ == 0:
                    ot_a = ps_ot.tile([HD + 1, QC], fp32, tag="ot", name="ot_a")
                    ot_b = ps_ot.tile([HD + 1, QC], fp32, tag="ot", name="ot_b")
                    ots[pc] = (ot_a, ot_b)
                ot_a, ot_b = ots[pc]
                nc.tensor.matmul(
                    ot_a, v_sb[:, pt_, :], ptile[:, 0:QC],
                    start=(pt_ == 0), stop=(pt_ == NKT - 1),
                )
                nc.tensor.matmul(
                    ot_b, v_sb[:, pt_, :], ptile[:, QC:2 * QC],
                    start=(pt_ == 0), stop=(pt_ == NKT - 1),
                )

            def emit_otcp(pc):
                # unnormalized O^T -> SBUF; denominators -> den_sb staging
                ot_a, ot_b = ots.pop(pc)
                nc.vector.tensor_copy(
                    ot_full[0:HD, bass.ts(pc, QC)], ot_a[0:HD, :]
                )
                nc.vector.tensor_copy(
                    ot_full[HD:DKV, bass.ts(pc, QC)], ot_b[0:HD, :]
                )
                nc.vector.tensor_copy(
                    den_sb[0:1, 0, pc, :], ot_a[HD:HD + 1, :]
                )
                nc.vector.tensor_copy(
                    den_sb[0:1, 1, pc, :], ot_b[HD:HD + 1, :]
                )

            dtr_ps = {}

            def emit_dtr_a(pc):
                dps = ps_tr.tile([128, 8], fp32, tag="tr", name="dps")
                dtr_ps[pc] = dps
                for u in range(2):
                    for h in range(2):
                        nc.tensor.transpose(
                            dps[:, 2 * u + h:2 * u + h + 1],
                            den_sb[0:1, h, pc, bass.ts(u, 128)],
                            identf[0:1, 0:1],
                        )

            def emit_dtr_b(pc):
                dps = dtr_ps.pop(pc)
                for u in range(2, 4):
                    for h in range(2):
                        nc.tensor.transpose(
                            dps[:, 2 * u + h:2 * u + h + 1],
                            den_sb[0:1, h, pc, bass.ts(u, 128)],
                            identf[0:1, 0:1],
                        )
                nc.vector.reciprocal(rsb_sb[:, pc, :], dps)

            def emit_y(pc, jq, u2):
                # outproj for q rows [pc*512+jq*128, +128), d cols u2*512:
                # per-head K=64 row-tiled pair, then normalize-and-sum on DVE
                yp_a = ps_tr.tile([128, MM_N], fp32, tag="tr", name="yp_a")
                yp_b = ps_tr.tile([128, MM_N], fp32, tag="tr", name="yp_b")
                qcol = pc * QC + jq * KT
                nc.tensor.matmul(
                    yp_a, ot_full[0:HD, qcol:qcol + KT],
                    ow_sb[0:HD, bass.ts(u2, MM_N)],
                    start=True, stop=True,
                )
                nc.tensor.matmul(
                    yp_b, ot_full[HD:DKV, qcol:qcol + KT],
                    ow_sb[HD:DKV, bass.ts(u2, MM_N)],
                    start=True, stop=True,
                )
                tmp = ytmp_pool.tile([128, MM_N], fp32, tag="yt", name="ytmp")
                nc.vector.tensor_scalar_mul(
                    tmp, yp_b, rsb_sb[:, pc, 2 * jq + 1:2 * jq + 2],
                )
                ysb = ysb_pool.tile([128, MM_N], bf16, tag="ysb", name="ysb")
                nc.vector.scalar_tensor_tensor(
                    ysb, yp_a, rsb_sb[:, pc, 2 * jq:2 * jq + 1],
                    tmp, MULT, ADD,
                )
                nc.sync.dma_start(
                    out=y[:, :][qcol:qcol + KT, bass.ts(u2, MM_N)], in_=ysb,
                )

            # ---------------- filler schedule ----------------
            # fill64[(c,t)]: 64x128-mode work, right after that step's
            # scores (Y outproj pairs). fill128[(c,t)]: full-array work
            # (projections, V transposes, denom transposes), before the
            # step's PV so the PE chews it while the previous exp drains.
            fill64 = {}
            fill128 = {}

            def add64(c, t, fn):
                fill64.setdefault((c, t), []).append(fn)

            def add128(c, t, fn):
                fill128.setdefault((c, t), []).append(fn)

            # chunk 0: kv slices 2-7 JIT (slice j spans windows 4j-8..4j-4,
            # done before its k-tiles are needed at step 4j), q1 at w26-30.
            for j in range(2, 8):
                st_ = {}
                w0 = 4 * j - 8
                add128(0, w0, lambda st_=st_, j=j: (
                    proj_mms(st_, kvw_sb, j, 0), proj_mms(st_, kvw_sb, j, 1)))
                add128(0, w0 + 1, lambda st_=st_, j=j: (
                    proj_mms(st_, kvw_sb, j, 2), proj_mms(st_, kvw_sb, j, 3)))
                add128(0, w0 + 3, lambda st_=st_, j=j: (
                    proj_fin(st_, kvt_sb, kvb_sb, j), kt2_dup(j),
                    v_transpose(4 * j), v_transpose(4 * j + 1)))
                add128(0, w0 + 4, lambda j=j: (
                    v_transpose(4 * j + 2), v_transpose(4 * j + 3)))
            qst = {}
            for k in range(4):
                add128(0, 26 + k, lambda k=k: proj_mms(qst, qw_sb, 1, k))
            add128(0, 30, lambda: proj_fin(qst, qt_sb, qb_sb, 1))

            # chunks >= 1: denom transposes w1-2, Y events w4..18 even,
            # next q slice w20-24
            for c in range(1, NCH):
                pc = c - 1
                add128(c, 1, lambda pc=pc: emit_dtr_a(pc))
                add128(c, 2, lambda pc=pc: emit_dtr_b(pc))
                for i in range(8):
                    add64(
                        c, 4 + 2 * i,
                        lambda pc=pc, jq=i // 2, u2=i % 2: emit_y(pc, jq, u2),
                    )
                if c <= 6:
                    qst_c = {}
                    for k in range(4):
                        add128(
                            c, 20 + k,
                            lambda d=qst_c, j=c + 1, k=k: proj_mms(d, qw_sb, j, k),
                        )
                    add128(
                        c, 24,
                        lambda d=qst_c, j=c + 1: proj_fin(d, qt_sb, qb_sb, j),
                    )

            # ---------------- prologue ----------------
            kv_slice_full(0)
            kv_slice_full(1)
            q_slice_full(0)

            # ---------------- main loop ----------------
            prev = None
            for step in range(NCH * NKT):
                c, t = divmod(step, NKT)
                pt = emit_scores(c, t)
                for fn in fill64.get((c, t), ()):
                    fn()
                for fn in fill128.get((c, t), ()):
                    fn()
                if prev is not None:
                    (pc, pt_), pptile = prev
                    emit_pv(pc, pt_, pptile)
                    if pt_ == NKT - 1:
                        emit_otcp(pc)
                prev = ((c, t), pt)

            # ---------------- epilogue ----------------
            (pc, pt_), pptile = prev
            emit_pv(pc, pt_, pptile)
            emit_otcp(pc)
            emit_dtr_a(pc)
            emit_dtr_b(pc)
            for i in range(8):
                emit_y(pc, i // 2, i % 2)
    _split_multi_waits(nc)
    return nc


def _split_multi_waits(nc):
    """This toolchain's walrus accepts at most one sync-wait per datapath
    instruction; move extra waits onto same-engine NoOps placed just before."""
    k = 0
    for f in nc.m.functions:
        for blk in f.blocks:
            out = []
            for inst in blk.instructions:
                si = getattr(inst, "sync_info", None)
                ow_ = list(si.on_wait) if (si and si.on_wait) else []
                if len(ow_) > 1:
                    for w in ow_[:-1]:
                        k += 1
                        nop = bass_rust.InstNoOp(
                            name=f"I-wsplit-{k}", ins=[], outs=[]
                        )
                        nop.engine = inst.engine
                        nop.sync_info = mybir.SyncInfo(
                            on_wait=[w], on_update=[]
                        )
                        out.append(nop)
                    inst.sync_info = mybir.SyncInfo(
                        on_wait=[ow_[-1]], on_update=list(si.on_update or [])
                    )
                out.append(inst)
            blk.instructions = out


def _prep_inputs(X, q_w, q_b, k_w, k_b, v_w, v_b, o_w):
    Xt = np.ascontiguousarray(X.reshape(S, D).T).astype(BF16)
    in_maps = []
    for c in range(NCORES):
        kv = c // (NCORES // KVH)
        qs = slice(c * DQ, (c + 1) * DQ)
        ks = slice(kv * HD, (kv + 1) * HD)
        in_maps.append({
            "xt": Xt,
            "qw": np.ascontiguousarray(q_w[:, qs]).astype(BF16),
            "kvw": np.ascontiguousarray(
                np.concatenate([k_w[:, ks], v_w[:, ks]], axis=1)).astype(BF16),
            "ow": np.ascontiguousarray(o_w[qs, :]).astype(BF16),
            "qb": np.ascontiguousarray(q_b[qs]).reshape(DQ, 1).astype(
                np.float32),
            "kvb": np.ascontiguousarray(
                np.concatenate([k_b[ks], v_b[ks]])).reshape(DKV, 1).astype(
                np.float32),
        })
    return in_maps


def kernel(X, q_w, q_b, k_w, k_b, v_w, v_b, o_w, o_b, **run_kwargs):
    global _COMPILED
    if _COMPILED is None:
        _COMPILED = build_bass()
    in_maps = _prep_inputs(X, q_w, q_b, k_w, k_b, v_w, v_b, o_w)
    res = run_bass_kernel_spmd(
        _COMPILED, in_maps, list(range(NCORES)), **run_kwargs
    )
    parts = [r["y"] for r in res.results]
    out = parts[0].astype(np.float32)
    for p in parts[1:]:
        out = out + p
    out = out + o_b.astype(np.float32)[None, :]
    if run_kwargs:
        return out.reshape(B, S, D), res
    return out.reshape(B, S, D)
